# revision 21
# baseline (speedup 1.0000x reference)
"""Trainium2 Bass kernel for nn_LossRegressionGaussianWithCorrelations.

Loss = (1/50) * (lp_var - lp_prior) - lp_lik with
  lp_prior = sum(-0.5*noisy^2 - 0.5*log2pi) / 512
  lp_lik   = sum(-0.5*((mu_pred - y)/sigma)^2 - log(sigma) - 0.5*log2pi) / 512
  lp_var   = sum_s(-0.5*(1024*log2pi + logdet(Sigma) + maha_s)) / 512,
             maha_s = dx_s^T Sigma^-1 dx_s,  dx = noisy - mu_w

Distribution (8 cores), final — fp8 stream, SBUF-traffic-minimized:
  - mu_prediction [65536,512] sharded along batch (8192 rows/core) and
    DOWNCAST TO FP8-E4M3 ON HOST (4MB/core; RNE quantization lands
    ~3e-4 relative on the loss vs the 2e-2 tolerance). fp8 products fit
    in <=8 mantissa bits, so the PE's e10m10 product path is exact.
  - Aggregate SBUF bandwidth is the binding resource when DMA + 4
    engines run together, so every pass minimizes SBUF bytes: fp8
    operands, fp8 scratch outputs, and no on-device tensors that the
    host can precompute.
  - sum(mu^2): grouped squares split ScalarE ACT (30 tiles, 1 read/elem)
    / VectorE STT (26 tiles, 2 reads/elem) / TensorE fp8 DoubleRow Gram
    chains (8 tiles; diagonals = column sums of squares, extracted with
    an identity-mask STT per 128-col slice). Each Gram slice chain owns
    a FULL PSUM bank tile: a start=True matmul pends the whole
    underlying PSUM tile, so interleaved chains must not share one
    (observed first-pair loss when they did).
  - cross term sum_b y_b r_b: fp8 DoubleRow matmuls y^T @ mu (2 K-tiles
    per pass, y padded to the 16B LDWEIGHTS stride) into one PSUM bank.
  - maha: dx = noisy - mu_w computed ON HOST and shipped as fp8; G via
    fp8 DoubleRow TensorE matmuls (2 sample-chunks per pass) into one
    fused PSUM tile; one fused maha STT (G * Sinv) on VectorE.
  - prior sum(noisy^2) and sum(y^2): host f64 (both are inputs).
  - Host does the O(n^3) inherently sequential part in fp64 (Cholesky ->
    logdet, inv) and the final fp64 combine of fp32 partials.
"""

import ml_dtypes
import numpy as np

BF16 = ml_dtypes.bfloat16
FP8 = ml_dtypes.float8_e4m3fn  # |x|<=240 bit-identical to TRN fp8e4

N_CORES = 8
P = 128          # partitions
BATCH = 65536
S = 512          # n_samples
W = 1024         # n_weights
RPC = BATCH // N_CORES   # batch rows per core = 8192
NT = RPC // P    # lik tiles per core (batch rows per partition) = 64
NC_CH = 8        # DMA chunks of 8 tiles (0.5MB fp8)
# square-engine per half-chunk (4 tiles)
HALF_ENG = ["act", "dve", "dve", "pool", "act", "gram", "act", "pool",
            "act", "pool", "gram", "pool", "act", "pool", "act", "act"]
NSQ = sum(1 for e in HALF_ENG if e != "gram") + 1  # accum slots (half 0 split)
NGQ = 4          # Gram 128-col slices
WA = S // P      # s-chunks of noisy = 4
NI = W // P      # i-chunks of G rows = 8
JC = W // N_CORES  # G/Sinv columns per core = 128

_STATE = {}


def _build_program():
    import concourse.bacc as bacc
    import concourse.bass as bass
    import concourse.mybir as mybir
    from concourse import tile

    f32 = mybir.dt.float32
    bf16 = mybir.dt.bfloat16
    fp8 = mybir.dt.float8e4
    nc = bacc.Bacc("TRN2", num_devices=N_CORES)

    mu = nc.dram_tensor("mu", [RPC, S], fp8, kind="ExternalInput").ap()
    # y padded to 16B element spacing: DoubleRow LDWEIGHTS requires the
    # outermost (K-tile pair) stride to be even and 16B-aligned
    y = nc.dram_tensor("y", [P, NT * 16], fp8, kind="ExternalInput").ap()
    dxin = nc.dram_tensor("dxin", [S, W], fp8, kind="ExternalInput").ap()
    dxcin = nc.dram_tensor("dxcin", [S, JC], fp8, kind="ExternalInput").ap()
    sinv = nc.dram_tensor("sinv", [W, JC], bf16, kind="ExternalInput").ap()
    ident = nc.dram_tensor("ident", [P, P], bf16, kind="ExternalInput").ap()
    NACC = NSQ + NGQ + 1
    out_acc = nc.dram_tensor("out_acc", [P, NACC], f32,
                             kind="ExternalOutput").ap()
    out_ym = nc.dram_tensor("out_ym", [1, S], f32, kind="ExternalOutput").ap()

    # batch row b = p*NT + t lives at partition p, tile t
    mu_v = mu.rearrange("(p t) s -> p t s", p=P)         # [128, 64, 512]
    dx_v = dxin.rearrange("(a p) w -> p a w", p=P)       # [128, 4, 1024]
    dxc_v = dxcin.rearrange("(a p) j -> p a j", p=P)     # [128, 4, 128]
    sinv_v = sinv.rearrange("(a p) j -> p a j", p=P)     # [128, 8, 128]

    with tile.TileContext(nc) as tc:
        with (
            tc.tile_pool(name="const", bufs=1) as const,
            tc.tile_pool(name="dump", bufs=2) as dumps,
            tc.tile_pool(name="ympsum", bufs=1, space="PSUM") as ympsum,
            tc.tile_pool(name="gpsum", bufs=1, space="PSUM") as gpsum,
            tc.tile_pool(name="scr", bufs=2) as scr,
        ):
            # sync ring carries almost everything (the SP engine has no
            # compute); the scalar ring only gets two late chunks, issued
            # between ACT squares so the ACT engine never stalls on a
            # backpressured ring before its first compute.
            y_sb = const.tile([P, NT, 16], fp8)
            nc.scalar.dma_start(out=y_sb,
                                in_=y.rearrange("p (t x) -> p t x", x=16))

            acc = const.tile([P, NACC], f32)
            acc_mu2 = acc[:, 0:NSQ]
            acc_gq = acc[:, NSQ:NSQ + NGQ]
            acc_maha = acc[:, NSQ + NGQ:]

            mu_sb = const.tile([P, NT, S], fp8)
            sinv_sb = const.tile([P, NI, JC], bf16)
            ident_sb = const.tile([P, P], bf16)

            ym = ympsum.tile([1, S], f32, tag="ym")
            # one FULL PSUM bank per Gram slice chain: a start=True matmul
            # pends the whole underlying PSUM tile, so interleaved chains
            # must not share one (observed first-pair loss when they did)
            gqs = [ympsum.tile([P, S], f32, tag=f"gq{c}", name=f"gq{c}")
                   for c in range(NGQ)]
            gg = gpsum.tile([P, NI, JC], f32, tag="gg")    # maha G banks
            dx_sb = const.tile([P, WA, W], fp8)
            dxc_sb = const.tile([P, WA, JC], fp8)

            def emit_g():
                # fp8 DoubleRow: 2 sample-chunks per pass, chains contiguous
                for i in range(NI):
                    for a in range(0, WA, 2):
                        nc.tensor.matmul(
                            out=gg[:, i, :],
                            lhsT=dx_sb[:, a:a + 2, i * P:(i + 1) * P],
                            rhs=dxc_sb[:, a:a + 2, :],
                            start=(a == 0), stop=(a == WA - 2),
                            perf_mode=mybir.MatmulPerfMode.DoubleRow,
                            skip_group_check=True)

            sq_slot = [0]
            gram_halves = [h for h, e in enumerate(HALF_ENG) if e == "gram"]
            first_gram, last_gram = gram_halves[0], gram_halves[-1]

            def emit_gram(h):
                j0 = 4 * h
                for t in range(j0, j0 + 4, 2):
                    for c in range(NGQ):
                        sl = mu_sb[:, t:t + 2, c * P:(c + 1) * P]
                        nc.tensor.matmul(
                            out=gqs[c][:, 0:P], lhsT=sl, rhs=sl,
                            start=(h == first_gram and t == j0),
                            stop=(h == last_gram and t == j0 + 2),
                            perf_mode=mybir.MatmulPerfMode.DoubleRow,
                            skip_group_check=True)

            def emit_sq(eng, t0, nt):
                if eng == "gram":
                    emit_gram(t0 // 4)
                    return
                flat = mu_sb[:, t0:t0 + nt, :].rearrange("p t s -> p (t s)")
                slot = sq_slot[0]
                sq_slot[0] += 1
                if eng == "act":
                    dummy = dumps.tile([P, nt * S], fp8, tag="dummy")
                    nc.scalar.activation(
                        out=dummy, in_=flat,
                        func=mybir.ActivationFunctionType.Square,
                        scale=1.0, bias=0.0,
                        accum_out=acc_mu2[:, slot:slot + 1])
                elif eng == "pool":
                    # GpSimd multiplies, DVE reduces: halves DVE's reads
                    sqp = scr.tile([P, nt * S], bf16, tag="sqp")
                    nc.gpsimd.tensor_mul(out=sqp, in0=flat, in1=flat)
                    nc.vector.tensor_reduce(
                        out=acc_mu2[:, slot:slot + 1], in_=sqp,
                        axis=mybir.AxisListType.X, op=mybir.AluOpType.add)
                else:
                    sq = scr.tile([P, nt * S], fp8, tag="sq")
                    nc.vector.scalar_tensor_tensor(
                        out=sq, in0=flat, scalar=1.0, in1=flat,
                        op0=mybir.AluOpType.mult, op1=mybir.AluOpType.mult,
                        accum_out=acc_mu2[:, slot:slot + 1])

            for ch in range(NC_CH):
                j0 = 8 * ch
                if ch == 0:
                    # split the first chunk so both engines start earliest
                    nc.sync.dma_start(out=mu_sb[:, 0:2, :], in_=mu_v[:, 0:2, :])
                    nc.sync.dma_start(out=mu_sb[:, 2:4, :], in_=mu_v[:, 2:4, :])
                    nc.sync.dma_start(out=mu_sb[:, 4:8, :], in_=mu_v[:, 4:8, :])
                    nc.sync.dma_start(out=dx_sb, in_=dx_v)
                    nc.sync.dma_start(out=dxc_sb, in_=dxc_v)
                    emit_sq("act", 0, 2)
                    emit_sq("dve", 2, 2)
                    emit_sq(HALF_ENG[1], 4, 4)
                else:
                    nc.sync.dma_start(out=mu_sb[:, j0:j0 + 8, :],
                                      in_=mu_v[:, j0:j0 + 8, :])
                    emit_sq(HALF_ENG[2 * ch], j0, 4)
                    emit_sq(HALF_ENG[2 * ch + 1], j0 + 4, 4)
                if ch == 1:
                    nc.sync.dma_start(out=sinv_sb, in_=sinv_v)
                    nc.sync.dma_start(out=ident_sb, in_=ident)
                # fp8 DoubleRow: one ym matmul consumes a pair of K-tiles
                for t in range(j0, j0 + 8, 2):
                    nc.tensor.matmul(
                        out=ym,
                        lhsT=y_sb[:, t:t + 2, 0:1],
                        rhs=mu_sb[:, t:t + 2, :],
                        start=(t == 0), stop=(t == NT - 2),
                        perf_mode=mybir.MatmulPerfMode.DoubleRow,
                        skip_group_check=True)
                if ch == 1:
                    emit_g()

            # Gram diag extraction: acc_gq[:, c] = sum_j gq_c[p, j]*I[p, j]
            for c in range(NGQ):
                gscr = scr.tile([P, P], f32, tag="gqscr")
                nc.vector.scalar_tensor_tensor(
                    out=gscr, in0=gqs[c][:, 0:P], scalar=1.0, in1=ident_sb,
                    op0=mybir.AluOpType.mult, op1=mybir.AluOpType.mult,
                    accum_out=acc_gq[:, c:c + 1])

            # fused maha contraction over all NI G-slices at once
            mscr = scr.tile([P, NI * JC], f32, tag="mscr")
            nc.vector.scalar_tensor_tensor(
                out=mscr, in0=gg.rearrange("p i j -> p (i j)"), scalar=1.0,
                in1=sinv_sb.rearrange("p i j -> p (i j)"),
                op0=mybir.AluOpType.mult, op1=mybir.AluOpType.mult,
                accum_out=acc_maha[:, 0:1])

            ym_sb = const.tile([1, S], f32)
            nc.vector.tensor_copy(out=ym_sb, in_=ym)
            nc.sync.dma_start(out=out_ym, in_=ym_sb)
            nc.sync.dma_start(out=out_acc, in_=acc)

    nc.compile()
    return nc


def _get_nc():
    if "nc" not in _STATE:
        _STATE["nc"] = _build_program()
    return _STATE["nc"]


def kernel(**inputs):
    noisy = np.ascontiguousarray(np.asarray(inputs["noisy_weights"], dtype=np.float32))
    mu_w = np.ascontiguousarray(np.asarray(inputs["mu_weights"], dtype=np.float32))
    Sigma = np.asarray(inputs["sigma_matrix_weights"])
    mu_p = np.ascontiguousarray(np.asarray(inputs["mu_prediction"], dtype=np.float32))
    sig_p = float(np.asarray(inputs["sigma_prediction"]))
    y = np.ascontiguousarray(np.asarray(inputs["y_true"], dtype=np.float32))

    # Host: the O(n^3) inherently-sequential factorization, in float64.
    S64 = Sigma.astype(np.float64)
    try:
        L = np.linalg.cholesky(S64)
    except np.linalg.LinAlgError:
        # jnp.linalg.cholesky yields NaNs for a non-SPD matrix, which
        # propagate to a NaN loss in the reference — match that.
        return np.float32(np.nan)
    logdet = 2.0 * float(np.sum(np.log(np.diagonal(L))))
    Sinv32 = np.linalg.inv(S64).astype(np.float32)

    nc = _get_nc()
    mu8 = mu_p.astype(FP8)
    y8 = y.astype(FP8)
    y8p = np.zeros((BATCH // NT, NT, 16), dtype=FP8)   # 16B-spaced for DoubleRow
    y8p[:, :, 0] = y8.reshape(BATCH // NT, NT)
    dx8 = (noisy - mu_w[None, :]).astype(FP8)          # host-side dx, fp8
    sinv16 = Sinv32.astype(BF16)
    ident16 = np.eye(P, dtype=BF16)
    in_maps = []
    for c in range(N_CORES):
        in_maps.append({
            "mu": mu8[c * RPC:(c + 1) * RPC],
            "y": y8p[c * P:(c + 1) * P].reshape(P, NT * 16),
            "dxin": dx8,
            "dxcin": np.ascontiguousarray(dx8[:, c * JC:(c + 1) * JC]),
            "sinv": np.ascontiguousarray(sinv16[:, c * JC:(c + 1) * JC]),
            "ident": ident16,
        })

    from concourse.bass_utils import run_bass_kernel_spmd
    res = run_bass_kernel_spmd(nc, in_maps, core_ids=list(range(N_CORES)))

    NACC = NSQ + NGQ + 1
    S_mu2 = float(sum(res.results[c]["out_acc"][:, 0:NSQ + NGQ]
                      .astype(np.float64).sum() for c in range(N_CORES)))
    S_yr = float(sum(res.results[c]["out_ym"].astype(np.float64).sum()
                     for c in range(N_CORES)))
    S_maha = float(sum(res.results[c]["out_acc"][:, NSQ + NGQ:NACC]
                       .astype(np.float64).sum() for c in range(N_CORES)))
    # exact-f64 host sums of the small input reductions
    S_y2 = float((y.astype(np.float64) ** 2).sum())
    S_pri = float((noisy.astype(np.float64) ** 2).sum())
    S_lik = S_mu2 - 2.0 * S_yr + S * S_y2

    log2pi = float(np.log(2.0 * np.pi))
    lp_prior = (-0.5 * S_pri - 0.5 * log2pi * (S * W)) / S
    lp_lik = (-0.5 * S_lik / (sig_p * sig_p)
              - (np.log(sig_p) + 0.5 * log2pi) * (BATCH * S)) / S
    lp_var = -0.5 * (S * W * log2pi + S * logdet + S_maha) / S
    total = (lp_var - lp_prior) / 50.0 - lp_lik
    return np.float32(total)


# revision 23
# speedup vs baseline: 1.1413x; 1.1413x over previous
"""Trainium2 Bass kernel for nn_LossRegressionGaussianWithCorrelations.

Loss = (1/50) * (lp_var - lp_prior) - lp_lik with
  lp_prior = sum(-0.5*noisy^2 - 0.5*log2pi) / 512
  lp_lik   = sum(-0.5*((mu_pred - y)/sigma)^2 - log(sigma) - 0.5*log2pi) / 512
  lp_var   = sum_s(-0.5*(1024*log2pi + logdet(Sigma) + maha_s)) / 512,
             maha_s = dx_s^T Sigma^-1 dx_s,  dx = noisy - mu_w

Distribution (8 cores), final — fp8 stream, SBUF-traffic-minimized:
  - mu_prediction [65536,512] sharded along batch (8192 rows/core) and
    DOWNCAST TO FP8-E4M3 ON HOST (4MB/core; RNE quantization lands
    ~3e-4 relative on the loss vs the 2e-2 tolerance). fp8 products fit
    in <=8 mantissa bits, so the PE's e10m10 product path is exact.
  - Aggregate SBUF bandwidth is the binding resource when DMA + 4
    engines run together, so every pass minimizes SBUF bytes: fp8
    operands, fp8 scratch outputs, and no on-device tensors that the
    host can precompute.
  - sum(mu^2): grouped squares split ScalarE ACT (30 tiles, 1 read/elem)
    / VectorE STT (26 tiles, 2 reads/elem) / TensorE fp8 DoubleRow Gram
    chains (8 tiles; diagonals = column sums of squares, extracted with
    an identity-mask STT per 128-col slice). Each Gram slice chain owns
    a FULL PSUM bank tile: a start=True matmul pends the whole
    underlying PSUM tile, so interleaved chains must not share one
    (observed first-pair loss when they did).
  - cross term sum_b y_b r_b: fp8 DoubleRow matmuls y^T @ mu (2 K-tiles
    per pass, y padded to the 16B LDWEIGHTS stride) into one PSUM bank.
  - maha: dx = noisy - mu_w computed ON HOST and shipped as fp8; G via
    fp8 DoubleRow TensorE matmuls (2 sample-chunks per pass) into one
    fused PSUM tile; one fused maha STT (G * Sinv) on VectorE.
  - prior sum(noisy^2) and sum(y^2): host f64 (both are inputs).
  - Host does the O(n^3) inherently sequential part in fp64 (Cholesky ->
    logdet, inv) and the final fp64 combine of fp32 partials.
"""

import ml_dtypes
import numpy as np

BF16 = ml_dtypes.bfloat16
FP8 = ml_dtypes.float8_e4m3fn  # |x|<=240 bit-identical to TRN fp8e4

N_CORES = 8
P = 128          # partitions
BATCH = 65536
S = 512          # n_samples
W = 1024         # n_weights
RPC = BATCH // N_CORES   # batch rows per core = 8192
NT = RPC // P    # lik tiles per core (batch rows per partition) = 64
NC_CH = 8        # DMA chunks of 8 tiles (0.5MB fp8)
# square-engine per half-chunk (4 tiles)
HALF_ENG = ["act", "dve", "act", "dve", "act", "gram", "act", "dve",
            "act", "dve", "gram", "dve", "act", "dve", "act", "act"]
NSQ = sum(1 for e in HALF_ENG if e != "gram") + 1  # accum slots (half 0 split)
NGQ = 4          # Gram 128-col slices
WA = S // P      # s-chunks of noisy = 4
NI = W // P      # i-chunks of G rows = 8
JC = W // N_CORES  # G/Sinv columns per core = 128

_STATE = {}


def _build_program():
    import concourse.bacc as bacc
    import concourse.bass as bass
    import concourse.mybir as mybir
    from concourse import tile

    f32 = mybir.dt.float32
    bf16 = mybir.dt.bfloat16
    fp8 = mybir.dt.float8e4
    nc = bacc.Bacc("TRN2", num_devices=N_CORES)

    mu = nc.dram_tensor("mu", [RPC, S], fp8, kind="ExternalInput").ap()
    # y padded to 16B element spacing: DoubleRow LDWEIGHTS requires the
    # outermost (K-tile pair) stride to be even and 16B-aligned
    y = nc.dram_tensor("y", [P, NT * 16], fp8, kind="ExternalInput").ap()
    dxin = nc.dram_tensor("dxin", [S, W], fp8, kind="ExternalInput").ap()
    dxcin = nc.dram_tensor("dxcin", [S, JC], fp8, kind="ExternalInput").ap()
    sinv = nc.dram_tensor("sinv", [W, JC], bf16, kind="ExternalInput").ap()
    ident = nc.dram_tensor("ident", [P, P], bf16, kind="ExternalInput").ap()
    NACC = NSQ + NGQ + 1
    out_acc = nc.dram_tensor("out_acc", [P, NACC], f32,
                             kind="ExternalOutput").ap()
    out_ym = nc.dram_tensor("out_ym", [1, S], f32, kind="ExternalOutput").ap()

    # batch row b = p*NT + t lives at partition p, tile t
    mu_v = mu.rearrange("(p t) s -> p t s", p=P)         # [128, 64, 512]
    dx_v = dxin.rearrange("(a p) w -> p a w", p=P)       # [128, 4, 1024]
    dxc_v = dxcin.rearrange("(a p) j -> p a j", p=P)     # [128, 4, 128]
    sinv_v = sinv.rearrange("(a p) j -> p a j", p=P)     # [128, 8, 128]

    with tile.TileContext(nc) as tc:
        with (
            tc.tile_pool(name="const", bufs=1) as const,
            tc.tile_pool(name="dump", bufs=2) as dumps,
            tc.tile_pool(name="ympsum", bufs=1, space="PSUM") as ympsum,
            tc.tile_pool(name="gpsum", bufs=1, space="PSUM") as gpsum,
            tc.tile_pool(name="scr", bufs=2) as scr,
        ):
            # sync ring carries almost everything (the SP engine has no
            # compute); the scalar ring only gets two late chunks, issued
            # between ACT squares so the ACT engine never stalls on a
            # backpressured ring before its first compute.
            y_sb = const.tile([P, NT, 16], fp8)

            acc = const.tile([P, NACC], f32)
            acc_mu2 = acc[:, 0:NSQ]
            acc_gq = acc[:, NSQ:NSQ + NGQ]
            acc_maha = acc[:, NSQ + NGQ:]

            mu_sb = const.tile([P, NT, S], fp8)
            sinv_sb = const.tile([P, NI, JC], bf16)
            ident_sb = const.tile([P, P], bf16)

            ym = ympsum.tile([1, S], f32, tag="ym")
            # one FULL PSUM bank per Gram slice chain: a start=True matmul
            # pends the whole underlying PSUM tile, so interleaved chains
            # must not share one (observed first-pair loss when they did)
            gqs = [ympsum.tile([P, S], f32, tag=f"gq{c}", name=f"gq{c}")
                   for c in range(NGQ)]
            gg = gpsum.tile([P, NI, JC], f32, tag="gg")    # maha G banks
            dx_sb = const.tile([P, WA, W], fp8)
            dxc_sb = const.tile([P, WA, JC], fp8)

            def emit_g():
                # fp8 DoubleRow: 2 sample-chunks per pass, chains contiguous
                for i in range(NI):
                    for a in range(0, WA, 2):
                        nc.tensor.matmul(
                            out=gg[:, i, :],
                            lhsT=dx_sb[:, a:a + 2, i * P:(i + 1) * P],
                            rhs=dxc_sb[:, a:a + 2, :],
                            start=(a == 0), stop=(a == WA - 2),
                            perf_mode=mybir.MatmulPerfMode.DoubleRow,
                            skip_group_check=True)

            sq_slot = [0]
            gram_halves = [h for h, e in enumerate(HALF_ENG) if e == "gram"]
            first_gram, last_gram = gram_halves[0], gram_halves[-1]

            def emit_gram(h):
                j0 = 4 * h
                for t in range(j0, j0 + 4, 2):
                    for c in range(NGQ):
                        sl = mu_sb[:, t:t + 2, c * P:(c + 1) * P]
                        nc.tensor.matmul(
                            out=gqs[c][:, 0:P], lhsT=sl, rhs=sl,
                            start=(h == first_gram and t == j0),
                            stop=(h == last_gram and t == j0 + 2),
                            perf_mode=mybir.MatmulPerfMode.DoubleRow,
                            skip_group_check=True)

            def emit_sq(eng, t0, nt):
                if eng == "gram":
                    emit_gram(t0 // 4)
                    return
                flat = mu_sb[:, t0:t0 + nt, :].rearrange("p t s -> p (t s)")
                slot = sq_slot[0]
                sq_slot[0] += 1
                if eng == "act":
                    dummy = dumps.tile([P, nt * S], fp8, tag="dummy")
                    nc.scalar.activation(
                        out=dummy, in_=flat,
                        func=mybir.ActivationFunctionType.Square,
                        scale=1.0, bias=0.0,
                        accum_out=acc_mu2[:, slot:slot + 1])
                else:
                    sq = scr.tile([P, nt * S], fp8, tag="sq")
                    nc.vector.scalar_tensor_tensor(
                        out=sq, in0=flat, scalar=1.0, in1=flat,
                        op0=mybir.AluOpType.mult, op1=mybir.AluOpType.mult,
                        accum_out=acc_mu2[:, slot:slot + 1])

            for ch in range(NC_CH):
                j0 = 8 * ch
                if ch == 0:
                    # split the first chunk so both engines start earliest
                    nc.sync.dma_start(out=mu_sb[:, 0:2, :], in_=mu_v[:, 0:2, :])
                    nc.sync.dma_start(out=mu_sb[:, 2:4, :], in_=mu_v[:, 2:4, :])
                    nc.sync.dma_start(out=mu_sb[:, 4:8, :], in_=mu_v[:, 4:8, :])
                    nc.sync.dma_start(
                        out=y_sb, in_=y.rearrange("p (t x) -> p t x", x=16))
                    nc.sync.dma_start(out=dx_sb, in_=dx_v)
                    nc.sync.dma_start(out=dxc_sb, in_=dxc_v)
                    emit_sq("act", 0, 2)
                    emit_sq("dve", 2, 2)
                    emit_sq(HALF_ENG[1], 4, 4)
                else:
                    nc.sync.dma_start(out=mu_sb[:, j0:j0 + 8, :],
                                      in_=mu_v[:, j0:j0 + 8, :])
                    emit_sq(HALF_ENG[2 * ch], j0, 4)
                    emit_sq(HALF_ENG[2 * ch + 1], j0 + 4, 4)
                if ch == 1:
                    nc.sync.dma_start(out=sinv_sb, in_=sinv_v)
                    nc.sync.dma_start(out=ident_sb, in_=ident)
                # fp8 DoubleRow: one ym matmul consumes a pair of K-tiles
                for t in range(j0, j0 + 8, 2):
                    nc.tensor.matmul(
                        out=ym,
                        lhsT=y_sb[:, t:t + 2, 0:1],
                        rhs=mu_sb[:, t:t + 2, :],
                        start=(t == 0), stop=(t == NT - 2),
                        perf_mode=mybir.MatmulPerfMode.DoubleRow,
                        skip_group_check=True)
                if ch == 1:
                    emit_g()

            # Gram diag extraction: acc_gq[:, c] = sum_j gq_c[p, j]*I[p, j]
            for c in range(NGQ):
                gscr = scr.tile([P, P], f32, tag="gqscr")
                nc.vector.scalar_tensor_tensor(
                    out=gscr, in0=gqs[c][:, 0:P], scalar=1.0, in1=ident_sb,
                    op0=mybir.AluOpType.mult, op1=mybir.AluOpType.mult,
                    accum_out=acc_gq[:, c:c + 1])

            # fused maha contraction over all NI G-slices at once
            mscr = scr.tile([P, NI * JC], f32, tag="mscr")
            nc.vector.scalar_tensor_tensor(
                out=mscr, in0=gg.rearrange("p i j -> p (i j)"), scalar=1.0,
                in1=sinv_sb.rearrange("p i j -> p (i j)"),
                op0=mybir.AluOpType.mult, op1=mybir.AluOpType.mult,
                accum_out=acc_maha[:, 0:1])

            ym_sb = const.tile([1, S], f32)
            nc.vector.tensor_copy(out=ym_sb, in_=ym)
            nc.sync.dma_start(out=out_ym, in_=ym_sb)
            nc.sync.dma_start(out=out_acc, in_=acc)

    nc.compile()
    return nc


def _get_nc():
    if "nc" not in _STATE:
        _STATE["nc"] = _build_program()
    return _STATE["nc"]


def kernel(**inputs):
    noisy = np.ascontiguousarray(np.asarray(inputs["noisy_weights"], dtype=np.float32))
    mu_w = np.ascontiguousarray(np.asarray(inputs["mu_weights"], dtype=np.float32))
    Sigma = np.asarray(inputs["sigma_matrix_weights"])
    mu_p = np.ascontiguousarray(np.asarray(inputs["mu_prediction"], dtype=np.float32))
    sig_p = float(np.asarray(inputs["sigma_prediction"]))
    y = np.ascontiguousarray(np.asarray(inputs["y_true"], dtype=np.float32))

    # Host: the O(n^3) inherently-sequential factorization, in float64.
    S64 = Sigma.astype(np.float64)
    try:
        L = np.linalg.cholesky(S64)
    except np.linalg.LinAlgError:
        # jnp.linalg.cholesky yields NaNs for a non-SPD matrix, which
        # propagate to a NaN loss in the reference — match that.
        return np.float32(np.nan)
    logdet = 2.0 * float(np.sum(np.log(np.diagonal(L))))
    Sinv32 = np.linalg.inv(S64).astype(np.float32)

    nc = _get_nc()
    mu8 = mu_p.astype(FP8)
    y8 = y.astype(FP8)
    y8p = np.zeros((BATCH // NT, NT, 16), dtype=FP8)   # 16B-spaced for DoubleRow
    y8p[:, :, 0] = y8.reshape(BATCH // NT, NT)
    dx8 = (noisy - mu_w[None, :]).astype(FP8)          # host-side dx, fp8
    sinv16 = Sinv32.astype(BF16)
    ident16 = np.eye(P, dtype=BF16)
    in_maps = []
    for c in range(N_CORES):
        in_maps.append({
            "mu": mu8[c * RPC:(c + 1) * RPC],
            "y": y8p[c * P:(c + 1) * P].reshape(P, NT * 16),
            "dxin": dx8,
            "dxcin": np.ascontiguousarray(dx8[:, c * JC:(c + 1) * JC]),
            "sinv": np.ascontiguousarray(sinv16[:, c * JC:(c + 1) * JC]),
            "ident": ident16,
        })

    from concourse.bass_utils import run_bass_kernel_spmd
    res = run_bass_kernel_spmd(nc, in_maps, core_ids=list(range(N_CORES)))

    NACC = NSQ + NGQ + 1
    S_mu2 = float(sum(res.results[c]["out_acc"][:, 0:NSQ + NGQ]
                      .astype(np.float64).sum() for c in range(N_CORES)))
    S_yr = float(sum(res.results[c]["out_ym"].astype(np.float64).sum()
                     for c in range(N_CORES)))
    S_maha = float(sum(res.results[c]["out_acc"][:, NSQ + NGQ:NACC]
                       .astype(np.float64).sum() for c in range(N_CORES)))
    # exact-f64 host sums of the small input reductions
    S_y2 = float((y.astype(np.float64) ** 2).sum())
    S_pri = float((noisy.astype(np.float64) ** 2).sum())
    S_lik = S_mu2 - 2.0 * S_yr + S * S_y2

    log2pi = float(np.log(2.0 * np.pi))
    lp_prior = (-0.5 * S_pri - 0.5 * log2pi * (S * W)) / S
    lp_lik = (-0.5 * S_lik / (sig_p * sig_p)
              - (np.log(sig_p) + 0.5 * log2pi) * (BATCH * S)) / S
    lp_var = -0.5 * (S * W * log2pi + S * logdet + S_maha) / S
    total = (lp_var - lp_prior) / 50.0 - lp_lik
    return np.float32(total)
